# revision 1
# baseline (speedup 1.0000x reference)
"""Trainium2 Bass kernel for the DDI DEDICOM decoder (nn_DDI_dedicom).

Reference computation (per edge a, relation b):
    x1 = x[edge[0]], x2 = x[edge[1]]                       # gather  [E, IN]
    row = BN(x1 @ W.T + b), col = BN(x2 @ W.T + b)         # linear + global-batch BN
    out[a, b] = sigmoid(row_a^T  diag(D_b) R diag(D_b)  col_a)

Sharding: data-parallel over E across 8 cores (E_s = E/8 = 4096 per core).
x / weights / R / D replicated. BatchNorm statistics are global over E:
each core computes per-feature partial sums (sum, sumsq) of its shard's
linear outputs; a [128,4] AllReduce produces the global stats.

Device layout is feature-major ([128 features on partitions, edges on free
dim]) so that the linear and the 16 DEDICOM matmuls contract features on
the PE, BN stats are free-axis reductions, and BN application is a
per-partition scale/bias.  Gathered edge-major [128e, 128f] tiles are
transposed on the PE.  The final per-edge dot (sum_i row*u) is an
elementwise DVE multiply + a PE "selector" matmul ([128,16] one-hot
column b) accumulating all 16 relations into one [16, 512] PSUM tile.
Output is produced relation-major [16, E_s]; the host transposes while
unsharding.
"""

import sys

sys.path.insert(0, "/opt/trn_rl_repo")

import numpy as np

import concourse.bass as bass
import concourse.tile as tile
from concourse import bacc, mybir
from concourse.bass_utils import run_bass_kernel_spmd

# Problem sizes (hardcoded per contract)
N_NODES = 50000
E = 32768
IN_DIM = 128
HID = 128
OUT = 16
EPS = 1e-5
N_CORES = 8
E_S = E // N_CORES          # 4096 edges per core
J = E_S // 128              # 32 gather blocks per side
NCH = E_S // 512            # 8 free-dim chunks of 512

F32 = mybir.dt.float32
F32R = mybir.dt.float32r

def _build(stage=3):
    """stage: 0=gather+linear, 1=+stats/cc/BN, 2=+dedicom(no out dbg), 3=full."""
    nc = bacc.Bacc(None, target_bir_lowering=False, debug=False, num_devices=N_CORES)

    # ---- I/O ----
    x = nc.dram_tensor("x", [N_NODES, IN_DIM], F32, kind="ExternalInput")
    idx1 = nc.dram_tensor("idx1", [128, J], mybir.dt.int32, kind="ExternalInput")
    idx2 = nc.dram_tensor("idx2", [128, J], mybir.dt.int32, kind="ExternalInput")
    w_t = nc.dram_tensor("w_t", [IN_DIM, HID], F32, kind="ExternalInput")
    r_t = nc.dram_tensor("r_t", [HID, HID], F32, kind="ExternalInput")
    d_m = nc.dram_tensor("d_m", [OUT, HID], F32, kind="ExternalInput")
    d_t = nc.dram_tensor("d_t", [HID, OUT], F32, kind="ExternalInput")
    lin_b = nc.dram_tensor("lin_b", [HID, 1], F32, kind="ExternalInput")
    gamma = nc.dram_tensor("gamma", [HID, 1], F32, kind="ExternalInput")
    beta = nc.dram_tensor("beta", [HID, 1], F32, kind="ExternalInput")
    ident = nc.dram_tensor("ident", [128, 128], F32, kind="ExternalInput")
    sel = nc.dram_tensor("sel", [128, OUT, OUT], F32, kind="ExternalInput")
    out = nc.dram_tensor("out", [OUT, E_S], F32, kind="ExternalOutput")
    if stage <= 1:
        row_dbg = nc.dram_tensor("row_dbg", [HID, E_S], F32, kind="ExternalOutput")
        col_dbg = nc.dram_tensor("col_dbg", [HID, E_S], F32, kind="ExternalOutput")

    with tile.TileContext(nc) as tc:
        with (
            tc.tile_pool(name="dramp", bufs=1, space="DRAM") as dramp,
            tc.tile_pool(name="consts", bufs=1) as consts,
            tc.tile_pool(name="gat", bufs=8) as gat,
            tc.tile_pool(name="big", bufs=1) as big,
            tc.tile_pool(name="zs", bufs=6) as zs,
            tc.tile_pool(name="small", bufs=2) as small,
            tc.tile_pool(name="outp", bufs=2) as outp,
            tc.tile_pool(name="psU", bufs=5, space="PSUM") as psU,
            tc.tile_pool(name="psO", bufs=3, space="PSUM") as psO,
        ):
            # ---- constants ----
            # idx first: the gather stream is the front-phase critical path
            idx1_s = consts.tile([128, J], mybir.dt.int32)
            nc.sync.dma_start(out=idx1_s[:], in_=idx1[:])
            idx2_s = consts.tile([128, J], mybir.dt.int32)
            nc.sync.dma_start(out=idx2_s[:], in_=idx2[:])
            w_t_s = consts.tile([IN_DIM, HID], F32)
            nc.sync.dma_start(out=w_t_s[:], in_=w_t[:])
            ident_s = consts.tile([128, 128], F32)
            nc.sync.dma_start(out=ident_s[:], in_=ident[:])
            r_t_s = consts.tile([HID, HID], F32)
            nc.sync.dma_start(out=r_t_s[:], in_=r_t[:])
            d_t_s = consts.tile([HID, OUT], F32)
            nc.sync.dma_start(out=d_t_s[:], in_=d_t[:])
            sel_s = consts.tile([128, OUT, OUT], F32)
            nc.sync.dma_start(out=sel_s[:], in_=sel[:])
            lin_b_s = consts.tile([HID, 1], F32)
            nc.sync.dma_start(out=lin_b_s[:], in_=lin_b[:])
            gamma_s = consts.tile([HID, 1], F32)
            nc.sync.dma_start(out=gamma_s[:], in_=gamma[:])
            beta_s = consts.tile([HID, 1], F32)
            nc.sync.dma_start(out=beta_s[:], in_=beta[:])
            # D broadcast across partitions: dbc[p, b, i] = D[b, i]
            dbc_s = consts.tile([128, OUT, HID], F32)
            nc.sync.dma_start(
                out=dbc_s[:],
                in_=bass.AP(tensor=d_m, offset=0, ap=[[0, 128], [HID, OUT], [1, HID]]),
            )
            eps_s = consts.tile([HID, 1], F32)
            nc.vector.memset(eps_s[:], EPS)
            # fp32r-rounded copy of the selector weights
            sel_r = consts.tile([128, OUT, OUT], F32R)
            nc.vector.tensor_copy(out=sel_r[:], in_=sel_s[:])
            # centered R^T: R~U(0,1); moving the 0.5*J rank-1 part to an
            # exact fp32 path shrinks the f32r residual magnitudes ~10x
            r_c = consts.tile([HID, HID], F32)
            nc.vector.tensor_scalar_add(out=r_c[:], in0=r_t_s[:], scalar1=-0.5)

            # ---- per-side gather + transpose + linear + stats ----
            # side 0: linear psum->sbuf copies on ACT with accum_out -> y sums.
            # side 1: transpose copies on ACT with accum_out -> x sums (the y
            # sum is then W @ xsum + E_s*b, one tiny matvec), linear copies on
            # DVE.  This keeps the last gather -> stats -> collective chain
            # short: nothing expensive serializes after the gather stream ends.
            yTs = []
            sum_parts = []
            sq_parts = []
            xsum_sb = []
            for side, idx_s in ((0, idx1_s), (1, idx2_s)):
                xT = big.tile([128, E_S], F32, tag=f"xT{side}")
                xs_part = small.tile([128, J], F32, tag=f"xs{side}")
                yT = big.tile([128, E_S], F32, tag=f"yT{side}")
                s_part = small.tile([128, NCH], F32, tag=f"sum{side}")
                q_part = small.tile([128, NCH], F32, tag=f"sq{side}")

                def emit_linear_chunk(n, side=side, xT=xT, yT=yT, s_part=s_part,
                                      q_part=q_part):
                    sl = slice(n * 512, (n + 1) * 512)
                    yp = psU.tile([128, 512], F32, tag="u")
                    nc.tensor.matmul(
                        out=yp[:], lhsT=w_t_s[:], rhs=xT[:, sl], start=True, stop=True
                    )
                    if side == 0:
                        # psum -> sbuf with bias add + free-axis sum
                        nc.scalar.activation(
                            out=yT[:, sl],
                            in_=yp[:],
                            func=mybir.ActivationFunctionType.Identity,
                            bias=lin_b_s[:, 0:1],
                            scale=1.0,
                            accum_out=(s_part[:, n : n + 1] if stage >= 1 else None),
                        )
                    else:
                        nc.vector.tensor_scalar_add(
                            out=yT[:, sl], in0=yp[:], scalar1=lin_b_s[:, 0:1]
                        )
                    if stage >= 1:
                        sq = zs.tile([128, 512], F32, tag="sq_scratch")
                        nc.scalar.activation(
                            out=sq[:],
                            in_=yT[:, sl],
                            func=mybir.ActivationFunctionType.Square,
                            accum_out=q_part[:, n : n + 1],
                        )

                for j in range(J):
                    g = gat.tile([128, 128], F32, tag="g")
                    nc.gpsimd.indirect_dma_start(
                        out=g[:],
                        out_offset=None,
                        in_=x[:],
                        in_offset=bass.IndirectOffsetOnAxis(
                            ap=idx_s[:, j : j + 1], axis=0
                        ),
                    )
                    tp = psO.tile([128, 128], F32, tag="o")
                    nc.tensor.transpose(out=tp[:], in_=g[:], identity=ident_s[:])
                    if side == 1 and stage >= 1:
                        nc.scalar.activation(
                            out=xT[:, j * 128 : (j + 1) * 128],
                            in_=tp[:],
                            func=mybir.ActivationFunctionType.Copy,
                            accum_out=xs_part[:, j : j + 1],
                        )
                    else:
                        nc.vector.tensor_copy(
                            out=xT[:, j * 128 : (j + 1) * 128], in_=tp[:]
                        )
                    # emit the linear for a 512-chunk as soon as its 4 blocks
                    # are in; keeps the linear off the post-gather tail
                    if j % 4 == 3:
                        emit_linear_chunk(j // 4)
                yTs.append(yT)
                sum_parts.append(s_part)
                sq_parts.append(q_part)
                if side == 1 and stage >= 1:
                    xs1 = small.tile([128, 1], F32, tag="xs1r")
                    nc.vector.reduce_sum(
                        out=xs1[:], in_=xs_part[:], axis=mybir.AxisListType.X,
                        op=mybir.AluOpType.add,
                    )
                    ysum_ps = psU.tile([128, 1], F32, tag="u")
                    nc.tensor.matmul(
                        out=ysum_ps[:], lhsT=w_t_s[:], rhs=xs1[:], start=True,
                        stop=True,
                    )
                    ysum1 = small.tile([128, 1], F32, tag="ys1")
                    nc.vector.scalar_tensor_tensor(
                        out=ysum1[:],
                        in0=lin_b_s[:, 0:1],
                        scalar=float(E_S),
                        in1=ysum_ps[:],
                        op0=mybir.AluOpType.mult,
                        op1=mybir.AluOpType.add,
                    )
                    xsum_sb.append(ysum1)

            if stage == 0:
                nc.sync.dma_start(out=row_dbg[:], in_=yTs[0][:])
                nc.sync.dma_start(out=col_dbg[:], in_=yTs[1][:])

            if stage >= 1:
                # ---- pack partial stats + AllReduce ----
                stats_l = small.tile([128, 4], F32, tag="stats")
                for k, part in ((0, sum_parts[0]), (1, sq_parts[0]), (3, sq_parts[1])):
                    nc.vector.reduce_sum(
                        out=stats_l[:, k : k + 1],
                        in_=part[:],
                        axis=mybir.AxisListType.X,
                        op=mybir.AluOpType.add,
                    )
                nc.vector.tensor_copy(out=stats_l[:, 2:3], in_=xsum_sb[0][:])
                cc_in = dramp.tile([HID, 4], F32)
                cc_out = dramp.tile([HID, 4], F32, addr_space="Shared")
                nc.sync.dma_start(out=cc_in[:], in_=stats_l[:])
                nc.gpsimd.collective_compute(
                    "AllReduce",
                    mybir.AluOpType.add,
                    replica_groups=[list(range(N_CORES))],
                    ins=[cc_in[:]],
                    outs=[cc_out[:]],
                )
                stats_g = small.tile([128, 4], F32, tag="statsg")
                nc.sync.dma_start(out=stats_g[:], in_=cc_out[:])

                # PE keep-warm across the collective: WAW-serialized dummy
                # transposes (each ~0.3us) so the HAM clock gate stays at
                # 8/8 and the post-collective matmuls start at full rate
                warm_ps = psU.tile([128, 128], F32, tag="u")
                for _k in range(90):
                    nc.tensor.transpose(
                        out=warm_ps[:], in_=yTs[0][:, 0:128], identity=ident_s[:]
                    )

                # ---- build S_b^T tiles (independent of stats; fills cc bubble)
                s_all = big.tile([128, OUT, HID], F32R, tag="s_all")
                for b in range(OUT):
                    nc.vector.tensor_tensor(
                        out=s_all[:, b, :],
                        in0=r_c[:],
                        in1=dbc_s[:, b, :],
                        op=mybir.AluOpType.mult,
                    )
                    nc.vector.tensor_scalar_mul(
                        out=s_all[:, b, :],
                        in0=s_all[:, b, :],
                        scalar1=d_t_s[:, b : b + 1],
                    )

                # ---- finalize BN factors ----
                inv_e = 1.0 / float(E)
                bn_s = []
                bn_t = []
                for side in (0, 1):
                    mean = small.tile([128, 1], F32, tag=f"m{side}")
                    nc.scalar.mul(
                        out=mean[:], in_=stats_g[:, 2 * side : 2 * side + 1], mul=inv_e
                    )
                    ey2 = small.tile([128, 1], F32, tag=f"e2{side}")
                    nc.scalar.mul(
                        out=ey2[:],
                        in_=stats_g[:, 2 * side + 1 : 2 * side + 2],
                        mul=inv_e,
                    )
                    var = small.tile([128, 1], F32, tag=f"v{side}")
                    nc.vector.tensor_tensor(
                        out=var[:], in0=mean[:], in1=mean[:], op=mybir.AluOpType.mult
                    )
                    nc.vector.tensor_sub(out=var[:], in0=ey2[:], in1=var[:])
                    std = small.tile([128, 1], F32, tag=f"sd{side}")
                    nc.scalar.activation(
                        out=std[:],
                        in_=var[:],
                        func=mybir.ActivationFunctionType.Sqrt,
                        bias=eps_s[:, 0:1],
                        scale=1.0,
                    )
                    inv = small.tile([128, 1], F32, tag=f"iv{side}")
                    nc.vector.reciprocal(out=inv[:], in_=std[:])
                    sc = small.tile([128, 1], F32, tag=f"sc{side}")
                    nc.vector.tensor_tensor(
                        out=sc[:], in0=gamma_s[:], in1=inv[:], op=mybir.AluOpType.mult
                    )
                    sh = small.tile([128, 1], F32, tag=f"sh{side}")
                    nc.vector.tensor_tensor(
                        out=sh[:], in0=mean[:], in1=sc[:], op=mybir.AluOpType.mult
                    )
                    nc.vector.tensor_sub(out=sh[:], in0=beta_s[:], in1=sh[:])
                    bn_s.append(sc)
                    bn_t.append(sh)

                # ---- apply BN (feature-major: per-partition scale+shift) ----
                rowT = big.tile([128, E_S], F32, tag="rowT")
                colT = big.tile([128, E_S], F32R, tag="colT")
                colF = big.tile([128, E_S], F32, tag="colF")
                for dst, src, side, eng in (
                    (rowT, yTs[0], 0, "act"),
                    (colT, yTs[1], 1, "dve"),
                    (colF, yTs[1], 1, "dve"),
                ):
                    for n in range(NCH):
                        sl = slice(n * 512, (n + 1) * 512)
                        if eng == "act":
                            nc.scalar.activation(
                                out=dst[:, sl],
                                in_=src[:, sl],
                                func=mybir.ActivationFunctionType.Identity,
                                bias=bn_t[side][:, 0:1],
                                scale=bn_s[side][:, 0:1],
                            )
                        else:
                            nc.vector.tensor_scalar(
                                out=dst[:, sl],
                                in0=src[:, sl],
                                scalar1=bn_s[side][:, 0:1],
                                scalar2=bn_t[side][:, 0:1],
                                op0=mybir.AluOpType.mult,
                                op1=mybir.AluOpType.add,
                            )

                if stage == 1:
                    nc.sync.dma_start(out=row_dbg[:], in_=rowT[:])
                    nc.sync.dma_start(out=col_dbg[:], in_=colT[:])

            if stage >= 2:
                # ---- exact rank-1 branch: v += 0.5 * (row.D_b)(D_b.col) ----
                # (R = 0.5*J + Rc; the 0.5*J part factorizes and is computed
                # here in full fp32 so the f32r residual path only carries
                # the small centered magnitudes)
                pq_sb = big.tile([OUT, E_S], F32, tag="pq")

                # ---- DEDICOM residual: u = Sc_b^T.T @ colT ; z = rowT*u ----
                # software-pipelined: u-matmuls + z-muls run G steps ahead of
                # the strictly-ordered o-accumulation matmuls so the z latency
                # (DVE/ACT/GPSIMD) stays off the PE's critical path.
                G = 3
                for n in range(NCH):
                    sl = slice(n * 512, (n + 1) * 512)
                    # exact rank-1 branch for this chunk (fp32)
                    p_ps = psO.tile([OUT, 512], F32, tag="o")
                    nc.tensor.matmul(
                        out=p_ps[:], lhsT=d_t_s[:], rhs=rowT[:, sl],
                        start=True, stop=True,
                    )
                    q_ps = psO.tile([OUT, 512], F32, tag="o")
                    nc.tensor.matmul(
                        out=q_ps[:], lhsT=d_t_s[:], rhs=colF[:, sl],
                        start=True, stop=True,
                    )
                    q_sb = outp.tile([OUT, 512], F32, tag="qsb")
                    nc.scalar.copy(out=q_sb[:], in_=q_ps[:])
                    nc.vector.scalar_tensor_tensor(
                        out=pq_sb[:, sl],
                        in0=p_ps[:],
                        scalar=0.5,
                        in1=q_sb[:],
                        op0=mybir.AluOpType.mult,
                        op1=mybir.AluOpType.mult,
                    )
                    op_ = psO.tile([OUT, 512], F32, tag="o")
                    ztiles = [None] * OUT

                    def emit_u_z(b):
                        up = psU.tile([128, 512], F32, tag="u")
                        nc.tensor.matmul(
                            out=up[:],
                            lhsT=s_all[:, b, :],
                            rhs=colT[:, sl],
                            start=True,
                            stop=True,
                        )
                        z = zs.tile([128, 512], F32R, tag="z")
                        if b % 16 < 6:
                            # third lane: ACT copies PSUM->SBUF, GPSIMD muls
                            u_sb = zs.tile([128, 512], F32, tag="usb")
                            nc.scalar.copy(out=u_sb[:], in_=up[:])
                            nc.gpsimd.tensor_tensor(
                                out=z[:],
                                in0=u_sb[:],
                                in1=rowT[:, sl],
                                op=mybir.AluOpType.mult,
                            )
                        else:
                            nc.vector.tensor_tensor(
                                out=z[:],
                                in0=up[:],
                                in1=rowT[:, sl],
                                op=mybir.AluOpType.mult,
                            )
                        ztiles[b] = z

                    def emit_o(b):
                        nc.tensor.matmul(
                            out=op_[:],
                            lhsT=sel_r[:, b, :],
                            rhs=ztiles[b][:],
                            start=(b == 0),
                            stop=(b == OUT - 1),
                        )

                    for b in range(OUT):
                        emit_u_z(b)
                        if b >= G:
                            emit_o(b - G)
                    for b in range(OUT - G, OUT):
                        emit_o(b)
                    o_mg = outp.tile([OUT, 512], F32, tag="omg")
                    nc.vector.tensor_add(
                        out=o_mg[:], in0=op_[:], in1=pq_sb[:, sl]
                    )
                    o_sb = outp.tile([OUT, 512], F32, tag="osb")
                    nc.scalar.activation(
                        out=o_sb[:],
                        in_=o_mg[:],
                        func=mybir.ActivationFunctionType.Sigmoid,
                    )
                    nc.sync.dma_start(out=out[:, sl], in_=o_sb[:])

    nc.compile()
    return nc


_CACHE = {}


def _get_nc():
    if "nc" not in _CACHE:
        _CACHE["nc"] = _build()
    return _CACHE["nc"]


def _marshal(x, target_edge_index, lin_w, lin_b, bn_gamma, bn_beta, R, D):
    x = np.ascontiguousarray(np.asarray(x, dtype=np.float32))
    edges = np.asarray(target_edge_index)
    sel = np.zeros((128, OUT, OUT), dtype=np.float32)
    for b in range(OUT):
        sel[:, b, b] = 1.0
    common = {
        "w_t": np.ascontiguousarray(np.asarray(lin_w, np.float32).T),
        "r_t": np.ascontiguousarray(np.asarray(R, np.float32).T),
        "d_m": np.ascontiguousarray(np.asarray(D, np.float32)),
        "d_t": np.ascontiguousarray(np.asarray(D, np.float32).T),
        "lin_b": np.ascontiguousarray(np.asarray(lin_b, np.float32).reshape(HID, 1)),
        "gamma": np.ascontiguousarray(np.asarray(bn_gamma, np.float32).reshape(HID, 1)),
        "beta": np.ascontiguousarray(np.asarray(bn_beta, np.float32).reshape(HID, 1)),
        "ident": np.eye(128, dtype=np.float32),
        "sel": sel,
        "x": x,
    }
    in_maps = []
    for c in range(N_CORES):
        sl = slice(c * E_S, (c + 1) * E_S)
        i1 = edges[0, sl].astype(np.int32).reshape(J, 128).T
        i2 = edges[1, sl].astype(np.int32).reshape(J, 128).T
        in_maps.append(
            {**common, "idx1": np.ascontiguousarray(i1), "idx2": np.ascontiguousarray(i2)}
        )
    return in_maps


def kernel(x, target_edge_index, lin_w, lin_b, bn_gamma, bn_beta, R, D):
    nc = _get_nc()
    in_maps = _marshal(x, target_edge_index, lin_w, lin_b, bn_gamma, bn_beta, R, D)
    _CACHE["in_maps"] = in_maps
    res = run_bass_kernel_spmd(nc, in_maps, list(range(N_CORES)))
    shards = [res.results[c]["out"] for c in range(N_CORES)]  # each [16, E_S]
    full = np.concatenate(shards, axis=1)  # [16, E]
    return np.ascontiguousarray(full.T)  # [E, 16] float32



# revision 22
# speedup vs baseline: 1.1267x; 1.1267x over previous
"""Trainium2 Bass kernel for the DDI DEDICOM decoder (nn_DDI_dedicom), v2.

Reference computation (per edge a, relation b):
    x1 = x[edge[0]], x2 = x[edge[1]]                       # gather  [E, IN]
    row = BN(x1 @ W.T + b), col = BN(x2 @ W.T + b)         # linear + global-batch BN
    out[a, b] = sigmoid(row_a^T  diag(D_b) R diag(D_b)  col_a)

Sharding: data-parallel over E across 8 cores (E_s = E/8 = 4096 per core).
x / weights replicated.  BatchNorm statistics are global over E.

v2 design vs the first kernel:
 - Gathers batched: 2 indirect DMAs per side (2048 descriptors each) instead
   of 64 x 128-desc issues; the per-issue SWDGE overhead dominated the front.
 - Stats exchange: AllGather of per-core [128,2] partials + local reduce
   (the cost model charges AllReduce 1.875x the AllGather constant).
 - Two pipelined AllGathers: col-side stats first (they gate the DEDICOM
   u-matmuls via the scaled rhs stream), row-side stats second (needed only
   by the selector weights and small rank-1 terms, ~12us later), so the
   collective latency hides under real PE work.
 - BatchNorm folded into the DEDICOM epilogue: with rowBN = scB*yB + shB,
   colBN = scA*yA + shA (y = raw linear outputs; the linear bias cancels
   through BN and is dropped), and R = Rc + 0.5*J (centering for f32r):
     score = sum_i scB_i * yB_i * u_b[i]          u_b = Sc_b @ (scA*yA)
           + (scB*c_b)^T yB + w3_b^T (scA*yA) + k0_b          [rank-1 shifts]
           + 0.5 * (p + pb)(q + qb)                           [0.5*J branch]
   c_b = Sc_b shA, w3_b = Sc_b^T shB, k0_b = c_b.shB, p = (scB*D)yB,
   q = D(scA*yA), pb = D shB, qb = D shA.  No BN application passes at all.
 - Everything on the f32r matmul path (1 cyc/row vs 4 for fp32).
 - s_all ([j,b,i] = Rc[i,j] D[b,i] D[b,j]) precomputed host-side (parameter
   preprocessing only, no edge/x data).
"""

import sys

sys.path.insert(0, "/opt/trn_rl_repo")

import numpy as np

import concourse.bass as bass
import concourse.tile as tile
from concourse import bacc, mybir
from concourse.bass_utils import run_bass_kernel_spmd

# Problem sizes (hardcoded per contract)
N_NODES = 50000
E = 32768
IN_DIM = 128
HID = 128
OUT = 16
EPS = 1e-5
N_CORES = 8
E_S = E // N_CORES          # 4096 edges per core
J = E_S // 128              # 32 gather blocks per side
NCH = E_S // 512            # 8 free-dim chunks of 512
NG = 2                      # gathers per side
JG = J // NG                # blocks per gather
DEL = 2                     # chunks the o-accumulation trails the u-matmuls
# z ring size: a z write in u-chunk n may only wait on o-chunks <= n-2 (which
# PE reaches right after u-chunk n); that requires ZBUFS >= 32.  40 = cushion.
ZBUFS = 36

F32 = mybir.dt.float32
F32R = mybir.dt.float32r
AF = mybir.ActivationFunctionType
ALU = mybir.AluOpType
AXX = mybir.AxisListType.X


def _build(stage=2):
    """stage: 0=gather+linear (y dbg), 1=+stats+BN factors (BN dbg), 2=full."""
    nc = bacc.Bacc(
        None,
        target_bir_lowering=False,
        debug=False,
        num_devices=N_CORES,
        dynamic_dma_scratch_size=1 << 16,
    )

    # ---- I/O ----
    # side A = col side (edge_index[1], contracted in u), B = row side.
    x = nc.dram_tensor("x", [N_NODES, IN_DIM], F32R, kind="ExternalInput")
    idxa = nc.dram_tensor("idxa", [128, J], mybir.dt.int32, kind="ExternalInput")
    idxb = nc.dram_tensor("idxb", [128, J], mybir.dt.int32, kind="ExternalInput")
    w_t = nc.dram_tensor("w_t", [IN_DIM, HID], F32R, kind="ExternalInput")
    rc_t = nc.dram_tensor("rc_t", [HID, HID], F32, kind="ExternalInput")  # (R-.5)^T
    rc_m = nc.dram_tensor("rc_m", [HID, HID], F32, kind="ExternalInput")  # R-.5
    d_t = nc.dram_tensor("d_t", [HID, OUT], F32, kind="ExternalInput")
    s_all_d = nc.dram_tensor("s_all", [HID, OUT, HID], F32R, kind="ExternalInput")
    gamma = nc.dram_tensor("gamma", [HID, 1], F32, kind="ExternalInput")
    beta = nc.dram_tensor("beta", [HID, 1], F32, kind="ExternalInput")
    ident = nc.dram_tensor("ident", [128, 128], F32R, kind="ExternalInput")
    out = nc.dram_tensor("out", [OUT, E_S], F32, kind="ExternalOutput")
    if stage <= 1:
        row_dbg = nc.dram_tensor("row_dbg", [HID, E_S], F32R, kind="ExternalOutput")
        col_dbg = nc.dram_tensor("col_dbg", [HID, E_S], F32R, kind="ExternalOutput")

    with tile.TileContext(nc) as tc:
        with (
            tc.tile_pool(name="dramp", bufs=1, space="DRAM") as dramp,
            tc.tile_pool(name="consts", bufs=1) as consts,
            tc.tile_pool(name="gat", bufs=1) as gat,
            tc.tile_pool(name="xtp", bufs=2) as xtp,
            tc.tile_pool(name="big", bufs=1) as big,
            tc.tile_pool(name="zs", bufs=ZBUFS) as zs,
            tc.tile_pool(name="usb", bufs=2) as usb,
            tc.tile_pool(name="small", bufs=2) as small,
            tc.tile_pool(name="outp", bufs=2) as outp,
            tc.tile_pool(name="qp", bufs=DEL + 1) as qp,
            tc.tile_pool(name="psU", bufs=4, space="PSUM") as psU,
            tc.tile_pool(name="psO", bufs=3, space="PSUM") as psO,
        ):
            # ---- constants (idx first: gathers are the front critical path)
            idxa_s = consts.tile([128, J], mybir.dt.int32)
            nc.sync.dma_start(out=idxa_s[:], in_=idxa[:])
            idxb_s = consts.tile([128, J], mybir.dt.int32)
            nc.sync.dma_start(out=idxb_s[:], in_=idxb[:])
            w_t_s = consts.tile([IN_DIM, HID], F32R)
            nc.sync.dma_start(out=w_t_s[:], in_=w_t[:])
            ident_s = consts.tile([128, 128], F32R)
            nc.sync.dma_start(out=ident_s[:], in_=ident[:])
            s_all = consts.tile([HID, OUT, HID], F32R)
            nc.sync.dma_start(out=s_all[:], in_=s_all_d[:])
            rc_t_s = consts.tile([HID, HID], F32)
            nc.sync.dma_start(out=rc_t_s[:], in_=rc_t[:])
            rc_m_s = consts.tile([HID, HID], F32)
            nc.sync.dma_start(out=rc_m_s[:], in_=rc_m[:])
            d_t_s = consts.tile([HID, OUT], F32)
            nc.sync.dma_start(out=d_t_s[:], in_=d_t[:])
            gamma_s = consts.tile([HID, 1], F32)
            nc.sync.dma_start(out=gamma_s[:], in_=gamma[:])
            beta_s = consts.tile([HID, 1], F32)
            nc.sync.dma_start(out=beta_s[:], in_=beta[:])
            eps_s = consts.tile([HID, 1], F32)
            nc.vector.memset(eps_s[:], EPS)
            d_t_r = consts.tile([HID, OUT], F32R)
            nc.vector.tensor_copy(out=d_t_r[:], in_=d_t_s[:])
            # preload the sqrt act table before the stats critical path
            sqrt_warm = consts.tile([128, 1], F32)
            nc.scalar.activation(
                out=sqrt_warm[:], in_=eps_s[:], func=AF.Sqrt, bias=0.0, scale=1.0
            )

            # ---- per-side gather + transpose + linear + stats partials ----
            ys = {}
            stats_sb = {}
            cc_outs = {}
            for side, idx_s in (("a", idxa_s), ("b", idxb_s)):
                y_t = big.tile([128, E_S], F32R, tag=f"y{side}")
                ysum_p = small.tile([128, NCH], F32, tag=f"ysum{side}")
                ysq_p = small.tile([128, NCH], F32, tag=f"ysq{side}")
                gts = []
                for h in range(NG):
                    gt = gat.tile([128, JG, 128], F32R, tag="g")
                    nc.gpsimd.indirect_dma_start(
                        out=gt[:],
                        out_offset=None,
                        in_=x[:],
                        in_offset=bass.IndirectOffsetOnAxis(
                            ap=idx_s[:, h * JG : (h + 1) * JG], axis=0
                        ),
                    )
                    gts.append(gt)
                for n in range(NCH):
                    xt = xtp.tile([128, 512], F32R, tag=f"xt{n % 2}")
                    for q in range(4):
                        j = n * 4 + q
                        tp = psO.tile([128, 128], F32R, tag="o")
                        nc.tensor.transpose(
                            out=tp[:], in_=gts[j // JG][:, j % JG, :],
                            identity=ident_s[:],
                        )
                        nc.vector.tensor_copy(
                            out=xt[:, q * 128 : (q + 1) * 128], in_=tp[:]
                        )
                    yp = psU.tile([128, 512], F32, tag="u")
                    nc.tensor.matmul(
                        out=yp[:], lhsT=w_t_s[:], rhs=xt[:], start=True, stop=True
                    )
                    sl = slice(n * 512, (n + 1) * 512)
                    nc.scalar.activation(
                        out=y_t[:, sl],
                        in_=yp[:],
                        func=AF.Identity,
                        bias=0.0,
                        scale=1.0,
                        accum_out=ysum_p[:, n : n + 1],
                    )
                    if stage >= 1:
                        sq = usb.tile([128, 512], F32, tag="usb")
                        nc.scalar.activation(
                            out=sq[:],
                            in_=yp[:],
                            func=AF.Square,
                            accum_out=ysq_p[:, n : n + 1],
                        )
                ys[side] = y_t
                if stage >= 1:
                    # pack [sum, sumsq] and AllGather partials across cores
                    st = small.tile([128, 2], F32, tag=f"st{side}")
                    nc.vector.reduce_sum(
                        out=st[:, 0:1], in_=ysum_p[:], axis=AXX, op=ALU.add
                    )
                    nc.vector.reduce_sum(
                        out=st[:, 1:2], in_=ysq_p[:], axis=AXX, op=ALU.add
                    )
                    cc_in = dramp.tile([HID, 2], F32)
                    cc_out = dramp.tile([N_CORES, HID, 2], F32, addr_space="Shared")
                    nc.sync.dma_start(out=cc_in[:], in_=st[:])
                    nc.gpsimd.collective_compute(
                        "AllGather",
                        ALU.bypass,
                        replica_groups=[list(range(N_CORES))],
                        ins=[cc_in[:]],
                        outs=[cc_out[:]],
                    )
                    cc_outs[side] = cc_out

            if stage == 0:
                nc.sync.dma_start(out=col_dbg[:], in_=ys["a"][:])
                nc.sync.dma_start(out=row_dbg[:], in_=ys["b"][:])

            def finalize(side):
                """global stats -> (sc, sh) for one side."""
                stg = small.tile([128, N_CORES, 2], F32, tag=f"stg{side}")
                nc.sync.dma_start(
                    out=stg[:], in_=cc_outs[side][:].rearrange("c p k -> p c k")
                )
                mean = small.tile([128, 1], F32, tag=f"m{side}")
                nc.vector.reduce_sum(out=mean[:], in_=stg[:, :, 0], axis=AXX,
                                     op=ALU.add)
                nc.vector.tensor_scalar_mul(out=mean[:], in0=mean[:],
                                            scalar1=1.0 / E)
                ey2 = small.tile([128, 1], F32, tag=f"e2{side}")
                nc.vector.reduce_sum(out=ey2[:], in_=stg[:, :, 1], axis=AXX,
                                     op=ALU.add)
                nc.vector.tensor_scalar_mul(out=ey2[:], in0=ey2[:], scalar1=1.0 / E)
                var = small.tile([128, 1], F32, tag=f"v{side}")
                nc.vector.tensor_tensor(out=var[:], in0=mean[:], in1=mean[:],
                                        op=ALU.mult)
                nc.vector.tensor_sub(out=var[:], in0=ey2[:], in1=var[:])
                std = small.tile([128, 1], F32, tag=f"sd{side}")
                nc.scalar.activation(out=std[:], in_=var[:], func=AF.Sqrt,
                                     bias=eps_s[:, 0:1], scale=1.0)
                inv = small.tile([128, 1], F32, tag=f"iv{side}")
                nc.vector.reciprocal(out=inv[:], in_=std[:])
                sc = small.tile([128, 1], F32, tag=f"sc{side}")
                nc.vector.tensor_tensor(out=sc[:], in0=gamma_s[:], in1=inv[:],
                                        op=ALU.mult)
                sh = small.tile([128, 1], F32, tag=f"sh{side}")
                nc.vector.tensor_tensor(out=sh[:], in0=mean[:], in1=sc[:],
                                        op=ALU.mult)
                nc.vector.tensor_sub(out=sh[:], in0=beta_s[:], in1=sh[:])
                return sc, sh

            if stage >= 1:
                scA, shA = finalize("a")

                # ---- side-A-dependent small builds ----
                # c[i,b] = (Sc_b shA)[i]; qb = D shA
                dshA = small.tile([HID, OUT], F32, tag="dshA")
                nc.vector.tensor_scalar_mul(out=dshA[:], in0=d_t_s[:],
                                            scalar1=shA[:, 0:1])
                m1_ps = psO.tile([HID, OUT], F32, tag="o")
                nc.tensor.matmul(out=m1_ps[:], lhsT=rc_t_s[:], rhs=dshA[:],
                                 start=True, stop=True)
                c_sb = small.tile([HID, OUT], F32, tag="c")
                nc.vector.tensor_tensor(out=c_sb[:], in0=m1_ps[:], in1=d_t_s[:],
                                        op=ALU.mult)
                qb_ps = psO.tile([OUT, 1], F32, tag="o")
                nc.tensor.matmul(out=qb_ps[:], lhsT=d_t_s[:], rhs=shA[:],
                                 start=True, stop=True)
                qb_h = small.tile([OUT, 1], F32, tag="qb")
                nc.vector.tensor_scalar_mul(out=qb_h[:], in0=qb_ps[:], scalar1=0.5)

                scB, shB = finalize("b")

                # ---- side-B-dependent small builds ----
                # selector weights: selw[:, b, m] = (m == b) * scB
                selw = consts.tile([128, OUT, OUT], F32R, tag="selw")
                nc.vector.memset(selw[:], 0.0)
                for b in range(OUT):
                    nc.vector.tensor_copy(out=selw[:, b, b : b + 1], in_=scB[:])
                scd_t = small.tile([HID, OUT], F32R, tag="scdt")
                nc.vector.tensor_scalar_mul(out=scd_t[:], in0=d_t_s[:],
                                            scalar1=scB[:, 0:1])
                c2_sb = small.tile([HID, OUT], F32R, tag="c2")
                nc.vector.tensor_scalar_mul(out=c2_sb[:], in0=c_sb[:],
                                            scalar1=scB[:, 0:1])
                dshB = small.tile([HID, OUT], F32, tag="dshB")
                nc.vector.tensor_scalar_mul(out=dshB[:], in0=d_t_s[:],
                                            scalar1=shB[:, 0:1])
                m2_ps = psO.tile([HID, OUT], F32, tag="o")
                nc.tensor.matmul(out=m2_ps[:], lhsT=rc_m_s[:], rhs=dshB[:],
                                 start=True, stop=True)
                w3_sb = small.tile([HID, OUT], F32R, tag="w3")
                nc.vector.tensor_tensor(out=w3_sb[:], in0=m2_ps[:], in1=d_t_s[:],
                                        op=ALU.mult)
                k0_ps = psO.tile([OUT, 1], F32, tag="o")
                nc.tensor.matmul(out=k0_ps[:], lhsT=c_sb[:], rhs=shB[:],
                                 start=True, stop=True)
                k0_sb = small.tile([OUT, 1], F32, tag="k0")
                nc.vector.tensor_copy(out=k0_sb[:], in_=k0_ps[:])
                pb_ps = psO.tile([OUT, 1], F32, tag="o")
                nc.tensor.matmul(out=pb_ps[:], lhsT=d_t_s[:], rhs=shB[:],
                                 start=True, stop=True)
                pb_sb = small.tile([OUT, 1], F32, tag="pb")
                nc.vector.tensor_copy(out=pb_sb[:], in_=pb_ps[:])

            if stage == 1:
                # debug: materialize BN'd row/col
                for dst, side, sc, sh in ((col_dbg, "a", scA, shA),
                                          (row_dbg, "b", scB, shB)):
                    dbg = big.tile([128, E_S], F32R, tag=f"dbg{side}")
                    for n in range(NCH):
                        sl = slice(n * 512, (n + 1) * 512)
                        nc.vector.tensor_scalar(
                            out=dbg[:, sl], in0=ys[side][:, sl],
                            scalar1=sc[:, 0:1], scalar2=sh[:, 0:1],
                            op0=ALU.mult, op1=ALU.add,
                        )
                    nc.sync.dma_start(out=dst[:], in_=dbg[:])

            if stage >= 2:
                ya_s = ys["a"]  # scaled in place chunk-by-chunk
                yB = ys["b"]
                ztiles = [[None] * OUT for _ in range(NCH)]
                qsbs = [None] * NCH

                def emit_u_chunk(n):
                    sl = slice(n * 512, (n + 1) * 512)
                    # scaled col stream for this chunk (gates on statsA only),
                    # in place over the raw linear output
                    nc.scalar.activation(
                        out=ya_s[:, sl], in_=ya_s[:, sl], func=AF.Copy,
                        bias=0.0, scale=scA[:, 0:1],
                    )
                    # q = D @ ya_s (+0.5 folding at copy), [16, 512]
                    q_ps = psO.tile([OUT, 512], F32, tag="o")
                    nc.tensor.matmul(out=q_ps[:], lhsT=d_t_r[:], rhs=ya_s[:, sl],
                                     start=True, stop=True)
                    q_sb = qp.tile([OUT, 512], F32, tag="qsb")
                    nc.scalar.activation(
                        out=q_sb[:], in_=q_ps[:], func=AF.Identity,
                        bias=qb_h[:, 0:1], scale=0.5,
                    )
                    qsbs[n] = q_sb
                    for b in range(OUT):
                        up = psU.tile([128, 512], F32, tag="u")
                        nc.tensor.matmul(
                            out=up[:], lhsT=s_all[:, b, :], rhs=ya_s[:, sl],
                            start=True, stop=True,
                        )
                        z = zs.tile([128, 512], F32R, tag="z")
                        if b % 2 == 0:
                            u_sb = usb.tile([128, 512], F32, tag="usb")
                            nc.scalar.copy(out=u_sb[:], in_=up[:])
                            nc.gpsimd.tensor_tensor(
                                out=z[:], in0=u_sb[:], in1=yB[:, sl], op=ALU.mult
                            )
                        else:
                            nc.vector.tensor_tensor(
                                out=z[:], in0=up[:], in1=yB[:, sl], op=ALU.mult
                            )
                        ztiles[n][b] = z

                def emit_o_chunk(m):
                    sl = slice(m * 512, (m + 1) * 512)
                    op_ = psO.tile([OUT, 512], F32, tag="o")
                    for b in range(OUT):
                        nc.tensor.matmul(
                            out=op_[:], lhsT=selw[:, b, :], rhs=ztiles[m][b][:],
                            start=(b == 0), stop=False,
                        )
                        ztiles[m][b] = None
                    nc.tensor.matmul(out=op_[:], lhsT=c2_sb[:], rhs=yB[:, sl],
                                     start=False, stop=False)
                    nc.tensor.matmul(out=op_[:], lhsT=w3_sb[:], rhs=ya_s[:, sl],
                                     start=False, stop=True)
                    p_ps = psO.tile([OUT, 512], F32, tag="o")
                    nc.tensor.matmul(out=p_ps[:], lhsT=scd_t[:], rhs=yB[:, sl],
                                     start=True, stop=True)
                    # pq = (p + pb) * (0.5 q + 0.5 qb)
                    pq = qp.tile([OUT, 512], F32, tag="pq")
                    nc.vector.scalar_tensor_tensor(
                        out=pq[:], in0=p_ps[:], scalar=pb_sb[:, 0:1],
                        in1=qsbs[m][:], op0=ALU.add, op1=ALU.mult,
                    )
                    o_mg = outp.tile([OUT, 512], F32, tag="omg")
                    nc.vector.tensor_add(out=o_mg[:], in0=op_[:], in1=pq[:])
                    o_sb = outp.tile([OUT, 512], F32, tag="osb")
                    nc.scalar.activation(
                        out=o_sb[:], in_=o_mg[:], func=AF.Sigmoid,
                        bias=k0_sb[:, 0:1], scale=1.0,
                    )
                    nc.sync.dma_start(out=out[:, sl], in_=o_sb[:])

                for n in range(NCH):
                    emit_u_chunk(n)
                    if n >= DEL:
                        emit_o_chunk(n - DEL)
                for m in range(NCH - DEL, NCH):
                    emit_o_chunk(m)
            elif stage <= 1:
                # dummy out so the output tensor exists
                o_sb = outp.tile([OUT, E_S], F32, tag="osb")
                nc.vector.memset(o_sb[:], 0.0)
                nc.sync.dma_start(out=out[:], in_=o_sb[:])

    nc.compile()
    return nc


_CACHE = {}


def _get_nc(stage=2):
    key = f"nc{stage}"
    if key not in _CACHE:
        _CACHE[key] = _build(stage)
    return _CACHE[key]


def _marshal(x, target_edge_index, lin_w, lin_b, bn_gamma, bn_beta, R, D):
    x = np.ascontiguousarray(np.asarray(x, dtype=np.float32))
    edges = np.asarray(target_edge_index)
    R = np.asarray(R, np.float64)
    D = np.asarray(D, np.float64)
    Rc = R - 0.5
    # s_all[j, b, i] = Rc[i, j] * D[b, i] * D[b, j]
    s_all = np.einsum('ij,bi,bj->jbi', Rc, D, D).astype(np.float32)
    common = {
        "x": x,
        "w_t": np.ascontiguousarray(np.asarray(lin_w, np.float32).T),
        "rc_t": np.ascontiguousarray(Rc.T.astype(np.float32)),
        "rc_m": np.ascontiguousarray(Rc.astype(np.float32)),
        "d_t": np.ascontiguousarray(D.T.astype(np.float32)),
        "s_all": np.ascontiguousarray(s_all),
        "gamma": np.ascontiguousarray(np.asarray(bn_gamma, np.float32).reshape(HID, 1)),
        "beta": np.ascontiguousarray(np.asarray(bn_beta, np.float32).reshape(HID, 1)),
        "ident": np.eye(128, dtype=np.float32),
    }
    in_maps = []
    for c in range(N_CORES):
        sl = slice(c * E_S, (c + 1) * E_S)
        ia = edges[1, sl].astype(np.int32).reshape(J, 128).T  # col side = A
        ib = edges[0, sl].astype(np.int32).reshape(J, 128).T  # row side = B
        in_maps.append(
            {**common, "idxa": np.ascontiguousarray(ia), "idxb": np.ascontiguousarray(ib)}
        )
    return in_maps


def kernel(x, target_edge_index, lin_w, lin_b, bn_gamma, bn_beta, R, D):
    nc = _get_nc()
    in_maps = _marshal(x, target_edge_index, lin_w, lin_b, bn_gamma, bn_beta, R, D)
    _CACHE["in_maps"] = in_maps
    res = run_bass_kernel_spmd(nc, in_maps, list(range(N_CORES)))
    shards = [res.results[c]["out"] for c in range(N_CORES)]  # each [16, E_S]
    full = np.concatenate(shards, axis=1)  # [16, E]
    return np.ascontiguousarray(full.T)  # [E, 16] float32


# revision 27
# speedup vs baseline: 1.2718x; 1.1288x over previous
"""Trainium2 Bass kernel for the DDI DEDICOM decoder (nn_DDI_dedicom), v2.

Reference computation (per edge a, relation b):
    x1 = x[edge[0]], x2 = x[edge[1]]                       # gather  [E, IN]
    row = BN(x1 @ W.T + b), col = BN(x2 @ W.T + b)         # linear + global-batch BN
    out[a, b] = sigmoid(row_a^T  diag(D_b) R diag(D_b)  col_a)

Sharding: data-parallel over E across 8 cores (E_s = E/8 = 4096 per core).
x / weights replicated.  BatchNorm statistics are global over E.

v2 design vs the first kernel:
 - Gathers batched: 2 indirect DMAs per side (2048 descriptors each) instead
   of 64 x 128-desc issues; the per-issue SWDGE overhead dominated the front.
 - Stats exchange: AllGather of per-core [128,2] partials + local reduce
   (the cost model charges AllReduce 1.875x the AllGather constant).
 - Two pipelined AllGathers: col-side stats first (they gate the DEDICOM
   u-matmuls via the scaled rhs stream), row-side stats second (needed only
   by the selector weights and small rank-1 terms, ~12us later), so the
   collective latency hides under real PE work.
 - BatchNorm folded into the DEDICOM epilogue: with rowBN = scB*yB + shB,
   colBN = scA*yA + shA (y = raw linear outputs; the linear bias cancels
   through BN and is dropped), and R = Rc + 0.5*J (centering for f32r):
     score = sum_i scB_i * yB_i * u_b[i]          u_b = Sc_b @ (scA*yA)
           + (scB*c_b)^T yB + w3_b^T (scA*yA) + k0_b          [rank-1 shifts]
           + 0.5 * (p + pb)(q + qb)                           [0.5*J branch]
   c_b = Sc_b shA, w3_b = Sc_b^T shB, k0_b = c_b.shB, p = (scB*D)yB,
   q = D(scA*yA), pb = D shB, qb = D shA.  No BN application passes at all.
 - Everything on the f32r matmul path (1 cyc/row vs 4 for fp32).
 - s_all ([j,b,i] = Rc[i,j] D[b,i] D[b,j]) precomputed host-side (parameter
   preprocessing only, no edge/x data).
"""

import sys

sys.path.insert(0, "/opt/trn_rl_repo")

import numpy as np

import concourse.bass as bass
import concourse.tile as tile
from concourse import bacc, mybir
from concourse.bass_utils import run_bass_kernel_spmd

# Problem sizes (hardcoded per contract)
N_NODES = 50000
E = 32768
IN_DIM = 128
HID = 128
OUT = 16
EPS = 1e-5
N_CORES = 8
E_S = E // N_CORES          # 4096 edges per core
J = E_S // 128              # 32 gather blocks per side
NCH = E_S // 512            # 8 free-dim chunks of 512
NG = 2                      # gathers per side
JG = J // NG                # blocks per gather
DEL = 2                     # chunks the o-accumulation trails the u-matmuls
# z ring size: a z write in u-chunk n may only wait on o-chunks <= n-2 (which
# PE reaches right after u-chunk n); that requires ZBUFS >= 32.  40 = cushion.
ZBUFS = 32

F32 = mybir.dt.float32
F32R = mybir.dt.float32r
AF = mybir.ActivationFunctionType
ALU = mybir.AluOpType
AXX = mybir.AxisListType.X


def _build(stage=2):
    """stage: 0=gather+linear (y dbg), 1=+stats+BN factors (BN dbg), 2=full."""
    nc = bacc.Bacc(
        None,
        target_bir_lowering=False,
        debug=False,
        num_devices=N_CORES,
        dynamic_dma_scratch_size=1 << 16,
    )

    # ---- I/O ----
    # side A = col side (edge_index[1], contracted in u), B = row side.
    x = nc.dram_tensor("x", [N_NODES, IN_DIM], F32R, kind="ExternalInput")
    idxa = nc.dram_tensor("idxa", [128, J], mybir.dt.int32, kind="ExternalInput")
    idxb = nc.dram_tensor("idxb", [128, J], mybir.dt.int32, kind="ExternalInput")
    w_t = nc.dram_tensor("w_t", [IN_DIM, HID], F32R, kind="ExternalInput")
    rc_t = nc.dram_tensor("rc_t", [HID, HID], F32, kind="ExternalInput")  # (R-.5)^T
    rc_m = nc.dram_tensor("rc_m", [HID, HID], F32, kind="ExternalInput")  # R-.5
    d_t = nc.dram_tensor("d_t", [HID, OUT], F32, kind="ExternalInput")
    s_all_d = nc.dram_tensor("s_all", [HID, OUT, HID], F32R, kind="ExternalInput")
    gamma = nc.dram_tensor("gamma", [HID, 1], F32, kind="ExternalInput")
    beta = nc.dram_tensor("beta", [HID, 1], F32, kind="ExternalInput")
    ident = nc.dram_tensor("ident", [128, 128], F32R, kind="ExternalInput")
    out = nc.dram_tensor("out", [OUT, E_S], F32, kind="ExternalOutput")
    if stage <= 1:
        row_dbg = nc.dram_tensor("row_dbg", [HID, E_S], F32R, kind="ExternalOutput")
        col_dbg = nc.dram_tensor("col_dbg", [HID, E_S], F32R, kind="ExternalOutput")

    with tile.TileContext(nc) as tc:
        with (
            tc.tile_pool(name="dramp", bufs=1, space="DRAM") as dramp,
            tc.tile_pool(name="consts", bufs=1) as consts,
            tc.tile_pool(name="gat", bufs=2) as gat,
            tc.tile_pool(name="xtp", bufs=2) as xtp,
            tc.tile_pool(name="big", bufs=1) as big,
            tc.tile_pool(name="zs", bufs=ZBUFS) as zs,
            tc.tile_pool(name="usb", bufs=2) as usb,
            tc.tile_pool(name="small", bufs=2) as small,
            tc.tile_pool(name="outp", bufs=2) as outp,
            tc.tile_pool(name="qp", bufs=DEL + 1) as qp,
            tc.tile_pool(name="psU", bufs=4, space="PSUM") as psU,
            tc.tile_pool(name="psO", bufs=3, space="PSUM") as psO,
        ):
            # ---- constants (idx first: gathers are the front critical path)
            idxa_s = consts.tile([128, J], mybir.dt.int32)
            nc.sync.dma_start(out=idxa_s[:], in_=idxa[:])
            idxb_s = consts.tile([128, J], mybir.dt.int32)
            nc.sync.dma_start(out=idxb_s[:], in_=idxb[:])
            w_t_s = consts.tile([IN_DIM, HID], F32R)
            nc.sync.dma_start(out=w_t_s[:], in_=w_t[:])
            ident_s = consts.tile([128, 128], F32R)
            nc.sync.dma_start(out=ident_s[:], in_=ident[:])
            s_all = consts.tile([HID, OUT, HID], F32R)
            nc.sync.dma_start(out=s_all[:], in_=s_all_d[:])
            rc_t_s = consts.tile([HID, HID], F32)
            nc.sync.dma_start(out=rc_t_s[:], in_=rc_t[:])
            rc_m_s = consts.tile([HID, HID], F32)
            nc.sync.dma_start(out=rc_m_s[:], in_=rc_m[:])
            d_t_s = consts.tile([HID, OUT], F32)
            nc.sync.dma_start(out=d_t_s[:], in_=d_t[:])
            gamma_s = consts.tile([HID, 1], F32)
            nc.sync.dma_start(out=gamma_s[:], in_=gamma[:])
            beta_s = consts.tile([HID, 1], F32)
            nc.sync.dma_start(out=beta_s[:], in_=beta[:])
            eps_s = consts.tile([HID, 1], F32)
            nc.vector.memset(eps_s[:], EPS)
            d_t_r = consts.tile([HID, OUT], F32R)
            nc.vector.tensor_copy(out=d_t_r[:], in_=d_t_s[:])
            # preload the sqrt act table before the stats critical path
            sqrt_warm = consts.tile([128, 1], F32)
            nc.scalar.activation(
                out=sqrt_warm[:], in_=eps_s[:], func=AF.Sqrt, bias=0.0, scale=1.0
            )

            # ---- gathers first: all 4 issue back-to-back on the Pool queue
            # (side-B's reuse side-A's buffers; the WAR dep on side-A's
            # transposes is data-tracked, the issue order stays compact)
            gtss = {}
            for side, idx_s in (("a", idxa_s), ("b", idxb_s)):
                gts = []
                for h in range(NG):
                    gt = gat.tile([128, JG, 128], F32R, tag="g")
                    nc.gpsimd.indirect_dma_start(
                        out=gt[:],
                        out_offset=None,
                        in_=x[:],
                        in_offset=bass.IndirectOffsetOnAxis(
                            ap=idx_s[:, h * JG : (h + 1) * JG], axis=0
                        ),
                    )
                    gts.append(gt)
                gtss[side] = gts

            # ---- per-side transpose + linear + stats partials ----
            ys = {}
            stats_st = {}
            cc_outs = {}
            for side in ("a", "b"):
                gts = gtss[side]
                y_t = big.tile([128, E_S], F32R, tag=f"y{side}")
                ysum_p = small.tile([128, NCH], F32, tag=f"ysum{side}")
                ysq_p = small.tile([128, NCH], F32, tag=f"ysq{side}")
                for n in range(NCH):
                    xt = xtp.tile([128, 512], F32R, tag=f"xt{n % 2}")
                    for q in range(4):
                        j = n * 4 + q
                        tp = psO.tile([128, 128], F32R, tag="o")
                        nc.tensor.transpose(
                            out=tp[:], in_=gts[j // JG][:, j % JG, :],
                            identity=ident_s[:],
                        )
                        nc.vector.tensor_copy(
                            out=xt[:, q * 128 : (q + 1) * 128], in_=tp[:]
                        )
                    yp = psU.tile([128, 512], F32, tag="u")
                    nc.tensor.matmul(
                        out=yp[:], lhsT=w_t_s[:], rhs=xt[:], start=True, stop=True
                    )
                    sl = slice(n * 512, (n + 1) * 512)
                    nc.scalar.activation(
                        out=y_t[:, sl],
                        in_=yp[:],
                        func=AF.Identity,
                        bias=0.0,
                        scale=1.0,
                        accum_out=ysum_p[:, n : n + 1],
                    )
                    if stage >= 1:
                        sq = usb.tile([128, 512], F32, tag="usb")
                        nc.scalar.activation(
                            out=sq[:],
                            in_=yp[:],
                            func=AF.Square,
                            accum_out=ysq_p[:, n : n + 1],
                        )
                ys[side] = y_t
                if stage >= 1:
                    # pack [sum, sumsq] partials; collectives are emitted later
                    st = small.tile([128, 2], F32, tag=f"st{side}")
                    nc.vector.reduce_sum(
                        out=st[:, 0:1], in_=ysum_p[:], axis=AXX, op=ALU.add
                    )
                    nc.vector.reduce_sum(
                        out=st[:, 1:2], in_=ysq_p[:], axis=AXX, op=ALU.add
                    )
                    stats_st[side] = st

            if stage >= 1:
                # AllGather the per-core stat partials; side A first (its
                # stats gate the u-matmuls), side B's lands ~15us later and
                # is only needed by the selector weights / rank-1 terms.
                for side in ("a", "b"):
                    cc_in = dramp.tile([HID, 2], F32)
                    cc_out = dramp.tile([N_CORES, HID, 2], F32, addr_space="Shared")
                    nc.sync.dma_start(out=cc_in[:], in_=stats_st[side][:])
                    nc.gpsimd.collective_compute(
                        "AllGather",
                        ALU.bypass,
                        replica_groups=[list(range(N_CORES))],
                        ins=[cc_in[:]],
                        outs=[cc_out[:]],
                    )
                    cc_outs[side] = cc_out

            if stage == 0:
                nc.sync.dma_start(out=col_dbg[:], in_=ys["a"][:])
                nc.sync.dma_start(out=row_dbg[:], in_=ys["b"][:])

            def finalize(side):
                """global stats -> (sc, sh) for one side."""
                stg = small.tile([128, N_CORES, 2], F32, tag=f"stg{side}")
                nc.sync.dma_start(
                    out=stg[:], in_=cc_outs[side][:].rearrange("c p k -> p c k")
                )
                mean = small.tile([128, 1], F32, tag=f"m{side}")
                nc.vector.reduce_sum(out=mean[:], in_=stg[:, :, 0], axis=AXX,
                                     op=ALU.add)
                nc.vector.tensor_scalar_mul(out=mean[:], in0=mean[:],
                                            scalar1=1.0 / E)
                ey2 = small.tile([128, 1], F32, tag=f"e2{side}")
                nc.vector.reduce_sum(out=ey2[:], in_=stg[:, :, 1], axis=AXX,
                                     op=ALU.add)
                nc.vector.tensor_scalar_mul(out=ey2[:], in0=ey2[:], scalar1=1.0 / E)
                var = small.tile([128, 1], F32, tag=f"v{side}")
                nc.vector.tensor_tensor(out=var[:], in0=mean[:], in1=mean[:],
                                        op=ALU.mult)
                nc.vector.tensor_sub(out=var[:], in0=ey2[:], in1=var[:])
                std = small.tile([128, 1], F32, tag=f"sd{side}")
                nc.scalar.activation(out=std[:], in_=var[:], func=AF.Sqrt,
                                     bias=eps_s[:, 0:1], scale=1.0)
                inv = small.tile([128, 1], F32, tag=f"iv{side}")
                nc.vector.reciprocal(out=inv[:], in_=std[:])
                sc = small.tile([128, 1], F32, tag=f"sc{side}")
                nc.vector.tensor_tensor(out=sc[:], in0=gamma_s[:], in1=inv[:],
                                        op=ALU.mult)
                sh = small.tile([128, 1], F32, tag=f"sh{side}")
                nc.vector.tensor_tensor(out=sh[:], in0=mean[:], in1=sc[:],
                                        op=ALU.mult)
                nc.vector.tensor_sub(out=sh[:], in0=beta_s[:], in1=sh[:])
                return sc, sh

            BB = {}  # statsB-dependent tiles, filled mid-pipeline

            def emit_statsA():
                scA, shA = finalize("a")
                # c[i,b] = (Sc_b shA)[i]; qb = D shA
                dshA = small.tile([HID, OUT], F32, tag="dshA")
                nc.vector.tensor_scalar_mul(out=dshA[:], in0=d_t_s[:],
                                            scalar1=shA[:, 0:1])
                m1_ps = psO.tile([HID, OUT], F32, tag="o")
                nc.tensor.matmul(out=m1_ps[:], lhsT=rc_t_s[:], rhs=dshA[:],
                                 start=True, stop=True)
                c_sb = small.tile([HID, OUT], F32, tag="c")
                nc.vector.tensor_tensor(out=c_sb[:], in0=m1_ps[:], in1=d_t_s[:],
                                        op=ALU.mult)
                qb_ps = psO.tile([OUT, 1], F32, tag="o")
                nc.tensor.matmul(out=qb_ps[:], lhsT=d_t_s[:], rhs=shA[:],
                                 start=True, stop=True)
                qb_h = small.tile([OUT, 1], F32, tag="qb")
                nc.vector.tensor_scalar_mul(out=qb_h[:], in0=qb_ps[:], scalar1=0.5)
                return scA, shA, c_sb, qb_h

            def emit_statsB(c_sb):
                scB, shB = finalize("b")
                # selector weights: selw[:, b, m] = (m == b) * scB
                selw = consts.tile([128, OUT, OUT], F32R, tag="selw")
                nc.vector.memset(selw[:], 0.0)
                for b in range(OUT):
                    nc.vector.tensor_copy(out=selw[:, b, b : b + 1], in_=scB[:])
                scd_t = small.tile([HID, OUT], F32R, tag="scdt")
                nc.vector.tensor_scalar_mul(out=scd_t[:], in0=d_t_s[:],
                                            scalar1=scB[:, 0:1])
                c2_sb = small.tile([HID, OUT], F32R, tag="c2")
                nc.vector.tensor_scalar_mul(out=c2_sb[:], in0=c_sb[:],
                                            scalar1=scB[:, 0:1])
                dshB = small.tile([HID, OUT], F32, tag="dshB")
                nc.vector.tensor_scalar_mul(out=dshB[:], in0=d_t_s[:],
                                            scalar1=shB[:, 0:1])
                m2_ps = psO.tile([HID, OUT], F32, tag="o")
                nc.tensor.matmul(out=m2_ps[:], lhsT=rc_m_s[:], rhs=dshB[:],
                                 start=True, stop=True)
                w3_sb = small.tile([HID, OUT], F32R, tag="w3")
                nc.vector.tensor_tensor(out=w3_sb[:], in0=m2_ps[:], in1=d_t_s[:],
                                        op=ALU.mult)
                k0_ps = psO.tile([OUT, 1], F32, tag="o")
                nc.tensor.matmul(out=k0_ps[:], lhsT=c_sb[:], rhs=shB[:],
                                 start=True, stop=True)
                k0_sb = small.tile([OUT, 1], F32, tag="k0")
                nc.vector.tensor_copy(out=k0_sb[:], in_=k0_ps[:])
                pb_ps = psO.tile([OUT, 1], F32, tag="o")
                nc.tensor.matmul(out=pb_ps[:], lhsT=d_t_s[:], rhs=shB[:],
                                 start=True, stop=True)
                pb_sb = small.tile([OUT, 1], F32, tag="pb")
                nc.vector.tensor_copy(out=pb_sb[:], in_=pb_ps[:])
                BB.update(scB=scB, shB=shB, selw=selw, scd_t=scd_t, c2=c2_sb,
                          w3=w3_sb, k0=k0_sb, pb=pb_sb)

            if stage >= 1:
                scA, shA, c_sb, qb_h = emit_statsA()

            if stage == 1:
                emit_statsB(c_sb)
                scB, shB = BB["scB"], BB["shB"]
                # debug: materialize BN'd row/col
                for dst, side, sc, sh in ((col_dbg, "a", scA, shA),
                                          (row_dbg, "b", scB, shB)):
                    dbg = big.tile([128, E_S], F32R, tag=f"dbg{side}")
                    for n in range(NCH):
                        sl = slice(n * 512, (n + 1) * 512)
                        nc.vector.tensor_scalar(
                            out=dbg[:, sl], in0=ys[side][:, sl],
                            scalar1=sc[:, 0:1], scalar2=sh[:, 0:1],
                            op0=ALU.mult, op1=ALU.add,
                        )
                    nc.sync.dma_start(out=dst[:], in_=dbg[:])

            if stage >= 2:
                ya_s = ys["a"]  # scaled in place chunk-by-chunk
                yB = ys["b"]
                ztiles = [[None] * OUT for _ in range(NCH)]
                qsbs = [None] * NCH

                def emit_u_chunk(n):
                    sl = slice(n * 512, (n + 1) * 512)
                    # scaled col stream for this chunk (gates on statsA only),
                    # in place over the raw linear output
                    nc.scalar.activation(
                        out=ya_s[:, sl], in_=ya_s[:, sl], func=AF.Copy,
                        bias=0.0, scale=scA[:, 0:1],
                    )
                    # q = D @ ya_s (+0.5 folding at copy), [16, 512]
                    q_ps = psO.tile([OUT, 512], F32, tag="o")
                    nc.tensor.matmul(out=q_ps[:], lhsT=d_t_r[:], rhs=ya_s[:, sl],
                                     start=True, stop=True)
                    q_sb = qp.tile([OUT, 512], F32, tag="qsb")
                    nc.scalar.activation(
                        out=q_sb[:], in_=q_ps[:], func=AF.Identity,
                        bias=qb_h[:, 0:1], scale=0.5,
                    )
                    qsbs[n] = q_sb
                    for b in range(OUT):
                        up = psU.tile([128, 512], F32, tag="u")
                        nc.tensor.matmul(
                            out=up[:], lhsT=s_all[:, b, :], rhs=ya_s[:, sl],
                            start=True, stop=True,
                        )
                        z = zs.tile([128, 512], F32R, tag="z")
                        if b % 2 == 0:
                            u_sb = usb.tile([128, 512], F32, tag="usb")
                            nc.scalar.copy(out=u_sb[:], in_=up[:])
                            nc.gpsimd.tensor_tensor(
                                out=z[:], in0=u_sb[:], in1=yB[:, sl], op=ALU.mult
                            )
                        else:
                            nc.vector.tensor_tensor(
                                out=z[:], in0=up[:], in1=yB[:, sl], op=ALU.mult
                            )
                        ztiles[n][b] = z

                def emit_o_chunk(m):
                    sl = slice(m * 512, (m + 1) * 512)
                    op_ = psO.tile([OUT, 512], F32, tag="o")
                    for b in range(OUT):
                        nc.tensor.matmul(
                            out=op_[:], lhsT=BB["selw"][:, b, :],
                            rhs=ztiles[m][b][:],
                            start=(b == 0), stop=False,
                        )
                        ztiles[m][b] = None
                    nc.tensor.matmul(out=op_[:], lhsT=BB["c2"][:], rhs=yB[:, sl],
                                     start=False, stop=False)
                    nc.tensor.matmul(out=op_[:], lhsT=BB["w3"][:],
                                     rhs=ya_s[:, sl], start=False, stop=True)
                    p_ps = psO.tile([OUT, 512], F32, tag="o")
                    nc.tensor.matmul(out=p_ps[:], lhsT=BB["scd_t"][:],
                                     rhs=yB[:, sl], start=True, stop=True)
                    # pq = (p + pb) * (0.5 q + 0.5 qb)
                    pq = outp.tile([OUT, 512], F32, tag="pq")
                    nc.vector.scalar_tensor_tensor(
                        out=pq[:], in0=p_ps[:], scalar=BB["pb"][:, 0:1],
                        in1=qsbs[m][:], op0=ALU.add, op1=ALU.mult,
                    )
                    o_mg = outp.tile([OUT, 512], F32, tag="omg")
                    nc.vector.tensor_add(out=o_mg[:], in0=op_[:], in1=pq[:])
                    o_sb = outp.tile([OUT, 512], F32, tag="osb")
                    nc.scalar.activation(
                        out=o_sb[:], in_=o_mg[:], func=AF.Sigmoid,
                        bias=BB["k0"][:, 0:1], scale=1.0,
                    )
                    nc.sync.dma_start(out=out[:, sl], in_=o_sb[:])

                # u-chunks 0..2 run on statsA alone; the statsB finalize +
                # builds are emitted after u-chunk 2 so the in-order DVE/ACT
                # queues reach them at about the time AllGather#2 lands.
                for n in range(NCH):
                    emit_u_chunk(n)
                    if n == DEL:
                        emit_statsB(c_sb)
                        emit_o_chunk(0)
                    elif n > DEL:
                        emit_o_chunk(n - DEL)
                for m in range(NCH - DEL, NCH):
                    emit_o_chunk(m)
            elif stage <= 1:
                # dummy out so the output tensor exists
                o_sb = outp.tile([OUT, E_S], F32, tag="osb")
                nc.vector.memset(o_sb[:], 0.0)
                nc.sync.dma_start(out=out[:], in_=o_sb[:])

    nc.compile()
    return nc


_CACHE = {}


def _get_nc(stage=2):
    key = f"nc{stage}"
    if key not in _CACHE:
        _CACHE[key] = _build(stage)
    return _CACHE[key]


def _marshal(x, target_edge_index, lin_w, lin_b, bn_gamma, bn_beta, R, D):
    x = np.ascontiguousarray(np.asarray(x, dtype=np.float32))
    edges = np.asarray(target_edge_index)
    R = np.asarray(R, np.float64)
    D = np.asarray(D, np.float64)
    Rc = R - 0.5
    # s_all[j, b, i] = Rc[i, j] * D[b, i] * D[b, j]
    s_all = np.einsum('ij,bi,bj->jbi', Rc, D, D).astype(np.float32)
    common = {
        "x": x,
        "w_t": np.ascontiguousarray(np.asarray(lin_w, np.float32).T),
        "rc_t": np.ascontiguousarray(Rc.T.astype(np.float32)),
        "rc_m": np.ascontiguousarray(Rc.astype(np.float32)),
        "d_t": np.ascontiguousarray(D.T.astype(np.float32)),
        "s_all": np.ascontiguousarray(s_all),
        "gamma": np.ascontiguousarray(np.asarray(bn_gamma, np.float32).reshape(HID, 1)),
        "beta": np.ascontiguousarray(np.asarray(bn_beta, np.float32).reshape(HID, 1)),
        "ident": np.eye(128, dtype=np.float32),
    }
    in_maps = []
    for c in range(N_CORES):
        sl = slice(c * E_S, (c + 1) * E_S)
        ia = edges[1, sl].astype(np.int32).reshape(J, 128).T  # col side = A
        ib = edges[0, sl].astype(np.int32).reshape(J, 128).T  # row side = B
        in_maps.append(
            {**common, "idxa": np.ascontiguousarray(ia), "idxb": np.ascontiguousarray(ib)}
        )
    return in_maps


def kernel(x, target_edge_index, lin_w, lin_b, bn_gamma, bn_beta, R, D):
    nc = _get_nc()
    in_maps = _marshal(x, target_edge_index, lin_w, lin_b, bn_gamma, bn_beta, R, D)
    _CACHE["in_maps"] = in_maps
    res = run_bass_kernel_spmd(nc, in_maps, list(range(N_CORES)))
    shards = [res.results[c]["out"] for c in range(N_CORES)]  # each [16, E_S]
    full = np.concatenate(shards, axis=1)  # [16, E]
    return np.ascontiguousarray(full.T)  # [E, 16] float32


# revision 37
# speedup vs baseline: 1.3316x; 1.0470x over previous
"""Trainium2 Bass kernel for the DDI DEDICOM decoder (nn_DDI_dedicom), v2.

Reference computation (per edge a, relation b):
    x1 = x[edge[0]], x2 = x[edge[1]]                       # gather  [E, IN]
    row = BN(x1 @ W.T + b), col = BN(x2 @ W.T + b)         # linear + global-batch BN
    out[a, b] = sigmoid(row_a^T  diag(D_b) R diag(D_b)  col_a)

Sharding: data-parallel over E across 8 cores (E_s = E/8 = 4096 per core).
x / weights replicated.  BatchNorm statistics are global over E.

v2 design vs the first kernel:
 - Gathers batched: 2 indirect DMAs per side (2048 descriptors each) instead
   of 64 x 128-desc issues; the per-issue SWDGE overhead dominated the front.
 - Stats exchange: AllGather of per-core [128,2] partials + local reduce
   (the cost model charges AllReduce 1.875x the AllGather constant).
 - Two pipelined AllGathers: col-side stats first (they gate the DEDICOM
   u-matmuls via the scaled rhs stream), row-side stats second (needed only
   by the selector weights and small rank-1 terms, ~12us later), so the
   collective latency hides under real PE work.
 - BatchNorm folded into the DEDICOM epilogue: with rowBN = scB*yB + shB,
   colBN = scA*yA + shA (y = raw linear outputs; the linear bias cancels
   through BN and is dropped), and R = Rc + 0.5*J (centering for f32r):
     score = sum_i scB_i * yB_i * u_b[i]          u_b = Sc_b @ (scA*yA)
           + (scB*c_b)^T yB + w3_b^T (scA*yA) + k0_b          [rank-1 shifts]
           + 0.5 * (p + pb)(q + qb)                           [0.5*J branch]
   c_b = Sc_b shA, w3_b = Sc_b^T shB, k0_b = c_b.shB, p = (scB*D)yB,
   q = D(scA*yA), pb = D shB, qb = D shA.  No BN application passes at all.
 - Everything on the f32r matmul path (1 cyc/row vs 4 for fp32).
 - s_all ([j,b,i] = Rc[i,j] D[b,i] D[b,j]) precomputed host-side (parameter
   preprocessing only, no edge/x data).
"""

import sys

sys.path.insert(0, "/opt/trn_rl_repo")

import numpy as np

import concourse.bass as bass
import concourse.tile as tile
from concourse import bacc, mybir
from concourse.bass_utils import run_bass_kernel_spmd

# Problem sizes (hardcoded per contract)
N_NODES = 50000
E = 32768
IN_DIM = 128
HID = 128
OUT = 16
EPS = 1e-5
N_CORES = 8
E_S = E // N_CORES          # 4096 edges per core
J = E_S // 128              # 32 gather blocks per side
NCH = E_S // 512            # 8 free-dim chunks of 512
NG = 2                      # gathers per side
JG = J // NG                # blocks per gather
DEL = 2                     # chunks the o-accumulation trails the u-matmuls
# z ring size: a z write in u-chunk n may only wait on o-chunks <= n-2 (which
# PE reaches right after u-chunk n); that requires ZBUFS >= 32.  40 = cushion.
ZBUFS = 32

F32 = mybir.dt.float32
F32R = mybir.dt.float32r
AF = mybir.ActivationFunctionType
ALU = mybir.AluOpType
AXX = mybir.AxisListType.X


def _build(stage=2):
    """stage: 0=gather+linear (y dbg), 1=+stats+BN factors (BN dbg), 2=full."""
    nc = bacc.Bacc(
        None,
        target_bir_lowering=False,
        debug=False,
        num_devices=N_CORES,
        dynamic_dma_scratch_size=1 << 16,
    )

    # ---- I/O ----
    # side A = col side (edge_index[1], contracted in u), B = row side.
    x = nc.dram_tensor("x", [N_NODES, IN_DIM], F32R, kind="ExternalInput")
    idxa = nc.dram_tensor("idxa", [128, J], mybir.dt.int32, kind="ExternalInput")
    idxb = nc.dram_tensor("idxb", [128, J], mybir.dt.int32, kind="ExternalInput")
    w_t = nc.dram_tensor("w_t", [IN_DIM, HID], F32R, kind="ExternalInput")
    w_m = nc.dram_tensor("w_m", [HID, IN_DIM], F32, kind="ExternalInput")
    rc_t = nc.dram_tensor("rc_t", [HID, HID], F32, kind="ExternalInput")  # (R-.5)^T
    rc_m = nc.dram_tensor("rc_m", [HID, HID], F32, kind="ExternalInput")  # R-.5
    d_t = nc.dram_tensor("d_t", [HID, OUT], F32, kind="ExternalInput")
    s_all_d = nc.dram_tensor("s_all", [HID, OUT, HID], F32R, kind="ExternalInput")
    gamma = nc.dram_tensor("gamma", [HID, 1], F32, kind="ExternalInput")
    beta = nc.dram_tensor("beta", [HID, 1], F32, kind="ExternalInput")
    ident = nc.dram_tensor("ident", [128, 128], F32R, kind="ExternalInput")
    out = nc.dram_tensor("out", [OUT, E_S], F32, kind="ExternalOutput")
    if stage <= 1:
        row_dbg = nc.dram_tensor("row_dbg", [HID, E_S], F32R, kind="ExternalOutput")
        col_dbg = nc.dram_tensor("col_dbg", [HID, E_S], F32R, kind="ExternalOutput")

    with tile.TileContext(nc) as tc:
        with (
            tc.tile_pool(name="dramp", bufs=1, space="DRAM") as dramp,
            tc.tile_pool(name="consts", bufs=1) as consts,
            tc.tile_pool(name="gat", bufs=2) as gat,
            tc.tile_pool(name="xtp", bufs=2) as xtp,
            tc.tile_pool(name="big", bufs=1) as big,
            tc.tile_pool(name="zs", bufs=ZBUFS) as zs,
            tc.tile_pool(name="usb", bufs=2) as usb,
            tc.tile_pool(name="small", bufs=2) as small,
            tc.tile_pool(name="outp", bufs=2) as outp,
            tc.tile_pool(name="qp", bufs=DEL + 1) as qp,
            tc.tile_pool(name="psU", bufs=4, space="PSUM") as psU,
            tc.tile_pool(name="psO", bufs=3, space="PSUM") as psO,
            tc.tile_pool(name="psC", bufs=1, space="PSUM") as psC,
        ):
            # ---- constants (idx first: gathers are the front critical path)
            idxa_s = consts.tile([128, J], mybir.dt.int32)
            nc.sync.dma_start(out=idxa_s[:], in_=idxa[:])
            idxb_s = consts.tile([128, J], mybir.dt.int32)
            nc.sync.dma_start(out=idxb_s[:], in_=idxb[:])
            w_t_s = consts.tile([IN_DIM, HID], F32R)
            nc.sync.dma_start(out=w_t_s[:], in_=w_t[:])
            w_m_s = consts.tile([HID, IN_DIM], F32)
            nc.sync.dma_start(out=w_m_s[:], in_=w_m[:])
            ident_s = consts.tile([128, 128], F32R)
            nc.sync.dma_start(out=ident_s[:], in_=ident[:])
            s_all = consts.tile([HID, OUT, HID], F32R)
            nc.sync.dma_start(out=s_all[:], in_=s_all_d[:])
            rc_t_s = consts.tile([HID, HID], F32)
            nc.sync.dma_start(out=rc_t_s[:], in_=rc_t[:])
            rc_m_s = consts.tile([HID, HID], F32)
            nc.sync.dma_start(out=rc_m_s[:], in_=rc_m[:])
            d_t_s = consts.tile([HID, OUT], F32)
            nc.sync.dma_start(out=d_t_s[:], in_=d_t[:])
            gamma_s = consts.tile([HID, 1], F32)
            nc.sync.dma_start(out=gamma_s[:], in_=gamma[:])
            beta_s = consts.tile([HID, 1], F32)
            nc.sync.dma_start(out=beta_s[:], in_=beta[:])
            eps_s = consts.tile([HID, 1], F32)
            nc.vector.memset(eps_s[:], EPS)
            d_t_r = consts.tile([HID, OUT], F32R)
            nc.vector.tensor_copy(out=d_t_r[:], in_=d_t_s[:])
            # preload the sqrt act table before the stats critical path
            sqrt_warm = consts.tile([128, 1], F32)
            nc.scalar.activation(
                out=sqrt_warm[:], in_=eps_s[:], func=AF.Sqrt, bias=0.0, scale=1.0
            )

            # ---- gathers first: all 4 issue back-to-back on the Pool queue
            # (side-B's reuse side-A's buffers; the WAR dep on side-A's
            # transposes is data-tracked, the issue order stays compact)
            gtss = {}
            for side, idx_s in (("a", idxa_s), ("b", idxb_s)):
                gts = []
                for h in range(NG):
                    gt = gat.tile([128, JG, 128], F32R, tag="g")
                    nc.gpsimd.indirect_dma_start(
                        out=gt[:],
                        out_offset=None,
                        in_=x[:],
                        in_offset=bass.IndirectOffsetOnAxis(
                            ap=idx_s[:, h * JG : (h + 1) * JG], axis=0
                        ),
                    )
                    gts.append(gt)
                gtss[side] = gts

            # ---- per-side transpose + linear + stats partials ----
            # Stats come from x, not y, so they skip the ACT stream entirely:
            #   sum_a y[f,a]  = (W @ sum_a x_a)[f]      (xsum accumulated on the
            #                                            xT copies' accum_out)
            #   sum_a y[f,a]^2 = diag(W C W^T)[f],  C = sum_a x_a x_a^T
            # C is accumulated on the PE from the raw edge-major gather blocks:
            # even block j contributes g_j^T g_j to the left half of a
            # [128,256] psum (rhs = [g_j | g_j+1]), odd j to the right half
            # (rhs = [g_j-1 | g_j]); the off-diagonal halves are discarded.
            ys = {}
            stats_st = {}
            cc_outs = {}

            def emit_cmm(cps, gts, j):
                jj = j % JG
                h = j // JG
                even = j % 2 == 0
                rl = slice(jj, jj + 2) if even else slice(jj - 1, jj + 1)
                ol = slice(0, 256) if even else slice(256, 512)
                nc.tensor.matmul(
                    out=cps[:, ol], lhsT=gts[h][:, jj, :], rhs=gts[h][:, rl, :],
                    start=(j < 2), stop=(j >= J - 2),
                )

            def emit_stats_tail(side, cps, xs_p):
                """xsum/C -> per-core [sum(y), sum(y^2)] partials."""
                xsum_f = small.tile([128, 1], F32, tag=f"xsf{side}")
                nc.vector.reduce_sum(out=xsum_f[:], in_=xs_p[:], axis=AXX,
                                     op=ALU.add)
                xsum = small.tile([128, 1], F32R, tag=f"xs{side}")
                nc.vector.tensor_copy(out=xsum[:], in_=xsum_f[:])
                ysum_ps = psO.tile([128, 1], F32, tag="o")
                nc.tensor.matmul(out=ysum_ps[:], lhsT=w_t_s[:], rhs=xsum[:],
                                 start=True, stop=True)
                # even chain: [C | junk] in cps[:, 0:256]; odd: [junk | C] in
                # cps[:, 256:512]
                c_sb = small.tile([128, 256], F32R, tag="csb")
                nc.vector.tensor_copy(out=c_sb[:, 0:128], in_=cps[:, 0:128])
                nc.vector.tensor_copy(out=c_sb[:, 128:256], in_=cps[:, 384:512])
                wc_ps = psO.tile([128, 256], F32, tag="o")
                nc.tensor.matmul(out=wc_ps[:], lhsT=w_t_s[:], rhs=c_sb[:],
                                 start=True, stop=True)
                scr = usb.tile([128, 512], F32, tag="usb")
                nc.vector.scalar_tensor_tensor(
                    out=scr[:, 0:128], in0=wc_ps[:, 0:128], scalar=1.0,
                    in1=wc_ps[:, 128:256], op0=ALU.mult, op1=ALU.add,
                )
                st = small.tile([128, 2], F32, tag=f"st{side}")
                nc.vector.scalar_tensor_tensor(
                    out=scr[:, 128:256], in0=scr[:, 0:128], scalar=1.0,
                    in1=w_m_s[:], op0=ALU.mult, op1=ALU.mult,
                    accum_out=st[:, 1:2],
                )
                nc.vector.tensor_copy(out=st[:, 0:1], in_=ysum_ps[:])
                stats_st[side] = st

            for side in ("a", "b"):
                gts = gtss[side]
                y_t = big.tile([128, E_S], F32R, tag=f"y{side}")
                xs_p = small.tile([128, NCH], F32, tag=f"xsp{side}")
                cps = psC.tile([128, 512], F32, tag="c")
                for n in range(NCH):
                    xt = xtp.tile([128, 512], F32R, tag=f"xt{n % 2}")
                    tp = psO.tile([128, 512], F32R, tag="o")
                    for q in range(4):
                        j = n * 4 + q
                        nc.tensor.transpose(
                            out=tp[:, q * 128 : (q + 1) * 128],
                            in_=gts[j // JG][:, j % JG, :],
                            identity=ident_s[:],
                        )
                    if stage >= 1:
                        nc.vector.tensor_scalar(
                            out=xt[:], in0=tp[:], scalar1=1.0, scalar2=None,
                            op0=ALU.mult, accum_out=xs_p[:, n : n + 1],
                        )
                    else:
                        nc.vector.tensor_copy(out=xt[:], in_=tp[:])
                    yp = psU.tile([128, 512], F32, tag="u")
                    nc.tensor.matmul(
                        out=yp[:], lhsT=w_t_s[:], rhs=xt[:], start=True, stop=True
                    )
                    sl = slice(n * 512, (n + 1) * 512)
                    nc.scalar.activation(
                        out=y_t[:, sl], in_=yp[:], func=AF.Copy,
                        bias=0.0, scale=1.0,
                    )
                    # side A interleaves the C matmuls (its stats gate AG#1);
                    # side B's trail its linears (its collective starts late
                    # anyway, once AG#1 releases the collective cores)
                    if stage >= 1 and side == "a":
                        for q in range(4):
                            j = n * 4 + q
                            emit_cmm(cps, gts, j)
                ys[side] = y_t
                if stage >= 1:
                    if side == "b":
                        for j in range(J):
                            emit_cmm(cps, gts, j)
                    emit_stats_tail(side, cps, xs_p)

            if stage >= 1:
                # AllGather the per-core stat partials; side A first (its
                # stats gate the u-matmuls), side B's lands ~15us later and
                # is only needed by the selector weights / rank-1 terms.
                for side in ("a", "b"):
                    cc_in = dramp.tile([HID, 2], F32)
                    cc_out = dramp.tile([N_CORES, HID, 2], F32, addr_space="Shared")
                    nc.sync.dma_start(out=cc_in[:], in_=stats_st[side][:])
                    nc.gpsimd.collective_compute(
                        "AllGather",
                        ALU.bypass,
                        replica_groups=[list(range(N_CORES))],
                        ins=[cc_in[:]],
                        outs=[cc_out[:]],
                    )
                    cc_outs[side] = cc_out

            if stage == 0:
                nc.sync.dma_start(out=col_dbg[:], in_=ys["a"][:])
                nc.sync.dma_start(out=row_dbg[:], in_=ys["b"][:])

            def finalize(side):
                """global stats -> (sc, sh) for one side."""
                stg = small.tile([128, N_CORES, 2], F32, tag=f"stg{side}")
                nc.sync.dma_start(
                    out=stg[:], in_=cc_outs[side][:].rearrange("c p k -> p c k")
                )
                mean = small.tile([128, 1], F32, tag=f"m{side}")
                nc.vector.reduce_sum(out=mean[:], in_=stg[:, :, 0], axis=AXX,
                                     op=ALU.add)
                nc.vector.tensor_scalar_mul(out=mean[:], in0=mean[:],
                                            scalar1=1.0 / E)
                ey2 = small.tile([128, 1], F32, tag=f"e2{side}")
                nc.vector.reduce_sum(out=ey2[:], in_=stg[:, :, 1], axis=AXX,
                                     op=ALU.add)
                nc.vector.tensor_scalar_mul(out=ey2[:], in0=ey2[:], scalar1=1.0 / E)
                var = small.tile([128, 1], F32, tag=f"v{side}")
                nc.vector.tensor_tensor(out=var[:], in0=mean[:], in1=mean[:],
                                        op=ALU.mult)
                nc.vector.tensor_sub(out=var[:], in0=ey2[:], in1=var[:])
                std = small.tile([128, 1], F32, tag=f"sd{side}")
                nc.scalar.activation(out=std[:], in_=var[:], func=AF.Sqrt,
                                     bias=eps_s[:, 0:1], scale=1.0)
                inv = small.tile([128, 1], F32, tag=f"iv{side}")
                nc.vector.reciprocal(out=inv[:], in_=std[:])
                sc = small.tile([128, 1], F32, tag=f"sc{side}")
                nc.vector.tensor_tensor(out=sc[:], in0=gamma_s[:], in1=inv[:],
                                        op=ALU.mult)
                sh = small.tile([128, 1], F32, tag=f"sh{side}")
                nc.vector.tensor_tensor(out=sh[:], in0=mean[:], in1=sc[:],
                                        op=ALU.mult)
                nc.vector.tensor_sub(out=sh[:], in0=beta_s[:], in1=sh[:])
                return sc, sh

            BB = {}  # statsB-dependent tiles, filled mid-pipeline

            def emit_statsA():
                scA, shA = finalize("a")
                # c[i,b] = (Sc_b shA)[i]; qb = D shA
                dshA = small.tile([HID, OUT], F32, tag="dshA")
                nc.vector.tensor_scalar_mul(out=dshA[:], in0=d_t_s[:],
                                            scalar1=shA[:, 0:1])
                m1_ps = psO.tile([HID, OUT], F32, tag="o")
                nc.tensor.matmul(out=m1_ps[:], lhsT=rc_t_s[:], rhs=dshA[:],
                                 start=True, stop=True)
                c_sb = small.tile([HID, OUT], F32, tag="c")
                nc.vector.tensor_tensor(out=c_sb[:], in0=m1_ps[:], in1=d_t_s[:],
                                        op=ALU.mult)
                qb_ps = psO.tile([OUT, 1], F32, tag="o")
                nc.tensor.matmul(out=qb_ps[:], lhsT=d_t_s[:], rhs=shA[:],
                                 start=True, stop=True)
                qb_h = small.tile([OUT, 1], F32, tag="qb")
                nc.vector.tensor_scalar_mul(out=qb_h[:], in0=qb_ps[:], scalar1=0.5)
                return scA, shA, c_sb, qb_h

            def emit_statsB(c_sb):
                scB, shB = finalize("b")
                # selector weights: selw[:, b, m] = (m == b) * scB
                selw = consts.tile([128, OUT, OUT], F32R, tag="selw")
                nc.vector.memset(selw[:], 0.0)
                for b in range(OUT):
                    nc.vector.tensor_copy(out=selw[:, b, b : b + 1], in_=scB[:])
                scd_t = small.tile([HID, OUT], F32R, tag="scdt")
                nc.vector.tensor_scalar_mul(out=scd_t[:], in0=d_t_s[:],
                                            scalar1=scB[:, 0:1])
                c2_sb = small.tile([HID, OUT], F32R, tag="c2")
                nc.vector.tensor_scalar_mul(out=c2_sb[:], in0=c_sb[:],
                                            scalar1=scB[:, 0:1])
                dshB = small.tile([HID, OUT], F32, tag="dshB")
                nc.vector.tensor_scalar_mul(out=dshB[:], in0=d_t_s[:],
                                            scalar1=shB[:, 0:1])
                m2_ps = psO.tile([HID, OUT], F32, tag="o")
                nc.tensor.matmul(out=m2_ps[:], lhsT=rc_m_s[:], rhs=dshB[:],
                                 start=True, stop=True)
                w3_sb = small.tile([HID, OUT], F32R, tag="w3")
                nc.vector.tensor_tensor(out=w3_sb[:], in0=m2_ps[:], in1=d_t_s[:],
                                        op=ALU.mult)
                k0_ps = psO.tile([OUT, 1], F32, tag="o")
                nc.tensor.matmul(out=k0_ps[:], lhsT=c_sb[:], rhs=shB[:],
                                 start=True, stop=True)
                k0_sb = small.tile([OUT, 1], F32, tag="k0")
                nc.vector.tensor_copy(out=k0_sb[:], in_=k0_ps[:])
                pb_ps = psO.tile([OUT, 1], F32, tag="o")
                nc.tensor.matmul(out=pb_ps[:], lhsT=d_t_s[:], rhs=shB[:],
                                 start=True, stop=True)
                pb_sb = small.tile([OUT, 1], F32, tag="pb")
                nc.vector.tensor_copy(out=pb_sb[:], in_=pb_ps[:])
                BB.update(scB=scB, shB=shB, selw=selw, scd_t=scd_t, c2=c2_sb,
                          w3=w3_sb, k0=k0_sb, pb=pb_sb)

            if stage >= 1:
                scA, shA, c_sb, qb_h = emit_statsA()

            if stage == 1:
                emit_statsB(c_sb)
                scB, shB = BB["scB"], BB["shB"]
                # debug: materialize BN'd row/col
                for dst, side, sc, sh in ((col_dbg, "a", scA, shA),
                                          (row_dbg, "b", scB, shB)):
                    dbg = big.tile([128, E_S], F32R, tag=f"dbg{side}")
                    for n in range(NCH):
                        sl = slice(n * 512, (n + 1) * 512)
                        nc.vector.tensor_scalar(
                            out=dbg[:, sl], in0=ys[side][:, sl],
                            scalar1=sc[:, 0:1], scalar2=sh[:, 0:1],
                            op0=ALU.mult, op1=ALU.add,
                        )
                    nc.sync.dma_start(out=dst[:], in_=dbg[:])

            if stage >= 2:
                ya_s = ys["a"]  # scaled in place chunk-by-chunk
                yB = ys["b"]
                ztiles = [[None] * OUT for _ in range(NCH)]
                qsbs = [None] * NCH

                def emit_u_chunk(n):
                    sl = slice(n * 512, (n + 1) * 512)
                    # scaled col stream for this chunk (gates on statsA only),
                    # in place over the raw linear output
                    nc.scalar.activation(
                        out=ya_s[:, sl], in_=ya_s[:, sl], func=AF.Copy,
                        bias=0.0, scale=scA[:, 0:1],
                    )
                    # q = D @ ya_s (+0.5 folding at copy), [16, 512]
                    q_ps = psO.tile([OUT, 512], F32, tag="o")
                    nc.tensor.matmul(out=q_ps[:], lhsT=d_t_r[:], rhs=ya_s[:, sl],
                                     start=True, stop=True)
                    q_sb = qp.tile([OUT, 512], F32, tag="qsb")
                    nc.scalar.activation(
                        out=q_sb[:], in_=q_ps[:], func=AF.Identity,
                        bias=qb_h[:, 0:1], scale=0.5,
                    )
                    qsbs[n] = q_sb
                    for b in range(OUT):
                        up = psU.tile([128, 512], F32, tag="u")
                        nc.tensor.matmul(
                            out=up[:], lhsT=s_all[:, b, :], rhs=ya_s[:, sl],
                            start=True, stop=True,
                        )
                        z = zs.tile([128, 512], F32R, tag="z")
                        if b % 2 == 0:
                            u_sb = usb.tile([128, 512], F32, tag="usb")
                            nc.scalar.copy(out=u_sb[:], in_=up[:])
                            nc.gpsimd.tensor_tensor(
                                out=z[:], in0=u_sb[:], in1=yB[:, sl], op=ALU.mult
                            )
                        else:
                            nc.vector.tensor_tensor(
                                out=z[:], in0=up[:], in1=yB[:, sl], op=ALU.mult
                            )
                        ztiles[n][b] = z

                def emit_o_chunk(m):
                    sl = slice(m * 512, (m + 1) * 512)
                    op_ = psO.tile([OUT, 512], F32, tag="o")
                    for b in range(OUT):
                        nc.tensor.matmul(
                            out=op_[:], lhsT=BB["selw"][:, b, :],
                            rhs=ztiles[m][b][:],
                            start=(b == 0), stop=False,
                        )
                        ztiles[m][b] = None
                    nc.tensor.matmul(out=op_[:], lhsT=BB["c2"][:], rhs=yB[:, sl],
                                     start=False, stop=False)
                    nc.tensor.matmul(out=op_[:], lhsT=BB["w3"][:],
                                     rhs=ya_s[:, sl], start=False, stop=True)
                    p_ps = psO.tile([OUT, 512], F32, tag="o")
                    nc.tensor.matmul(out=p_ps[:], lhsT=BB["scd_t"][:],
                                     rhs=yB[:, sl], start=True, stop=True)
                    # pq = (p + pb) * (0.5 q + 0.5 qb)
                    pq = outp.tile([OUT, 512], F32, tag="pq")
                    nc.vector.scalar_tensor_tensor(
                        out=pq[:], in0=p_ps[:], scalar=BB["pb"][:, 0:1],
                        in1=qsbs[m][:], op0=ALU.add, op1=ALU.mult,
                    )
                    o_mg = outp.tile([OUT, 512], F32, tag="omg")
                    nc.vector.tensor_add(out=o_mg[:], in0=op_[:], in1=pq[:])
                    o_sb = outp.tile([OUT, 512], F32, tag="osb")
                    nc.scalar.activation(
                        out=o_sb[:], in_=o_mg[:], func=AF.Sigmoid,
                        bias=BB["k0"][:, 0:1], scale=1.0,
                    )
                    nc.sync.dma_start(out=out[:, sl], in_=o_sb[:])

                # u-chunks 0..2 run on statsA alone; the statsB finalize +
                # builds are emitted after u-chunk 2 so the in-order DVE/ACT
                # queues reach them at about the time AllGather#2 lands.
                for n in range(NCH):
                    emit_u_chunk(n)
                    if n == DEL:
                        emit_statsB(c_sb)
                        emit_o_chunk(0)
                    elif n > DEL:
                        emit_o_chunk(n - DEL)
                for m in range(NCH - DEL, NCH):
                    emit_o_chunk(m)
            elif stage <= 1:
                # dummy out so the output tensor exists
                o_sb = outp.tile([OUT, E_S], F32, tag="osb")
                nc.vector.memset(o_sb[:], 0.0)
                nc.sync.dma_start(out=out[:], in_=o_sb[:])

    nc.compile()
    return nc


_CACHE = {}


def _get_nc(stage=2):
    key = f"nc{stage}"
    if key not in _CACHE:
        _CACHE[key] = _build(stage)
    return _CACHE[key]


def _marshal(x, target_edge_index, lin_w, lin_b, bn_gamma, bn_beta, R, D):
    x = np.ascontiguousarray(np.asarray(x, dtype=np.float32))
    edges = np.asarray(target_edge_index)
    R = np.asarray(R, np.float64)
    D = np.asarray(D, np.float64)
    Rc = R - 0.5
    # s_all[j, b, i] = Rc[i, j] * D[b, i] * D[b, j]
    s_all = np.einsum('ij,bi,bj->jbi', Rc, D, D).astype(np.float32)
    common = {
        "x": x,
        "w_t": np.ascontiguousarray(np.asarray(lin_w, np.float32).T),
        "rc_t": np.ascontiguousarray(Rc.T.astype(np.float32)),
        "rc_m": np.ascontiguousarray(Rc.astype(np.float32)),
        "d_t": np.ascontiguousarray(D.T.astype(np.float32)),
        "s_all": np.ascontiguousarray(s_all),
        "gamma": np.ascontiguousarray(np.asarray(bn_gamma, np.float32).reshape(HID, 1)),
        "beta": np.ascontiguousarray(np.asarray(bn_beta, np.float32).reshape(HID, 1)),
        "ident": np.eye(128, dtype=np.float32),
    }
    in_maps = []
    for c in range(N_CORES):
        sl = slice(c * E_S, (c + 1) * E_S)
        ia = edges[1, sl].astype(np.int32).reshape(J, 128).T  # col side = A
        ib = edges[0, sl].astype(np.int32).reshape(J, 128).T  # row side = B
        in_maps.append(
            {**common, "idxa": np.ascontiguousarray(ia), "idxb": np.ascontiguousarray(ib)}
        )
    return in_maps


def kernel(x, target_edge_index, lin_w, lin_b, bn_gamma, bn_beta, R, D):
    nc = _get_nc()
    in_maps = _marshal(x, target_edge_index, lin_w, lin_b, bn_gamma, bn_beta, R, D)
    _CACHE["in_maps"] = in_maps
    res = run_bass_kernel_spmd(nc, in_maps, list(range(N_CORES)))
    shards = [res.results[c]["out"] for c in range(N_CORES)]  # each [16, E_S]
    full = np.concatenate(shards, axis=1)  # [16, E]
    return np.ascontiguousarray(full.T)  # [E, 16] float32
